# revision 15
# baseline (speedup 1.0000x reference)
"""MultiHeadDiffAttention Trainium2 kernel (8 NeuronCores).

Sharding: batch (4) x head-group (2 groups of 8 heads) = 8 cores.
Each core computes a partial (T, C) c_proj output for its batch element
restricted to its 8 heads; the host sums the two head-group partials per
batch element.

Per-core pipeline (all matmuls on PE, fp32r for fp32 data, fp16 for the
attention probabilities / V):
  1. PE-transpose x[b] -> xT (C on partitions).
  2. Projections: Q1/Q2 and K1/K2 in (head_dim, T) layout (weights are
     host-interleaved so each 128-row chunk = one head's [q1|q2] dims);
     V in (T, vdim) layout with an appended ones column.
  3. Per head/stream: scores S^T(k,q) = K^T-tiles x Q^T (contract d=64),
     exp via ScalarE (fused 1/8 scale) -> fp16 P, causal mask via
     gpsimd memset/affine_select, then PV: P-tile^T x [V|1] accumulated
     over k-tiles gives Y and the softmax denominator in one matmul.
  4. Streams combined as z = Y1 - (lam*den1/den2) * Y2 (per-q scalars),
     which equals den1 * (a1 - lam*a2) @ V; LayerNorm is scale-invariant
     per row, so normalizing z with eps scaled by den1^2 reproduces the
     reference exactly.
  5. LN via bn_stats/bn_aggr + exp(-0.5*ln(var+eps*den1^2) + ln(1-li)).
  6. PE-transpose y_ln, c_proj vs host-sliced Wc rows -> partial out.
"""

import contextlib
import ctypes
import math
import sys
import types

import numpy as np

sys.path.insert(0, "/opt/trn_rl_repo")


def _install_ntff_hook():
    """Provide antenv.axon_hooks if the image lacks it (for trace=True)."""
    try:
        from antenv.axon_hooks import get_axon_ntff_profile_hook  # noqa: F401

        return
    except ImportError:
        pass

    so_path = "/opt/axon/libaxon_pjrt.so"

    def _make_hook():
        try:
            lib = ctypes.CDLL(so_path)
        except OSError:
            return None
        if not hasattr(lib, "axon_start_nrt_profile"):
            return None
        lib.axon_start_nrt_profile.argtypes = [
            ctypes.POINTER(ctypes.c_int64),
            ctypes.c_size_t,
        ]
        lib.axon_start_nrt_profile.restype = ctypes.c_int64
        lib.axon_stop_nrt_profile.argtypes = [ctypes.c_char_p]
        lib.axon_stop_nrt_profile.restype = ctypes.c_int64

        @contextlib.contextmanager
        def _hook(output_dir, device_ids):
            import jax

            jax.devices()
            if device_ids:
                ids = (ctypes.c_int64 * len(device_ids))(*device_ids)
                rc = lib.axon_start_nrt_profile(ids, len(device_ids))
            else:
                rc = lib.axon_start_nrt_profile(None, 0)
            if rc != 0:
                raise RuntimeError(f"axon_start_nrt_profile rc={rc}")
            try:
                yield
            finally:
                n = lib.axon_stop_nrt_profile(str(output_dir).encode())
                if n < 0:
                    raise RuntimeError(f"axon_stop_nrt_profile rc={n}")

        return _hook

    mod = types.ModuleType("antenv.axon_hooks")
    _the_hook = _make_hook()
    mod.get_axon_ntff_profile_hook = lambda: _the_hook
    sys.modules["antenv.axon_hooks"] = mod


_install_ntff_hook()

import concourse.bass as bass  # noqa: E402
import concourse.bass_utils as bass_utils_mod  # noqa: E402
import concourse.mybir as mybir  # noqa: E402
import concourse.tile as tile  # noqa: E402
from concourse.masks import make_identity  # noqa: E402


def _enable_ldw_opt():
    """walrus ships with --enable-ldw-opt=false hardcoded; the LDWEIGHTS
    optimization pass overlaps weight loads with in-flight matmuls, which
    matters a lot for this kernel (a fresh stationary operand per matmul
    in the PV stage). Rewrite the flag on the walrus command line."""
    orig = bass_utils_mod.run_command

    def patched(argv, **kwargs):
        argv = [
            "--enable-ldw-opt=true" if a == "--enable-ldw-opt=false" else a
            for a in argv
        ]
        return orig(argv, **kwargs)

    bass_utils_mod.run_command = patched


import os  # noqa: E402

if os.environ.get("BASS_LDW_OPT", "0") == "1":
    _enable_ldw_opt()

P = 128
T = 1024
C = 1024
NH = 8  # heads per core
HS = 64
LAMBDA_INIT = 0.8 - 0.6 * math.exp(-0.3 * (2 - 1))
LN_EPS = 1e-5
N_CORES = 8

f32 = mybir.dt.float32
f32r = mybir.dt.float32r
f16 = mybir.dt.float16
Alu = mybir.AluOpType
Act = mybir.ActivationFunctionType


def r(ap):
    return ap.bitcast(f32r)


def build_program():
    nc = bass.Bass()
    x_d = nc.dram_tensor("x", [T, C], f32, kind="ExternalInput")
    wq_d = nc.dram_tensor("wq", [C, C], f32r, kind="ExternalInput")
    wk_d = nc.dram_tensor("wk", [C, C], f32r, kind="ExternalInput")
    wv_d = nc.dram_tensor("wv", [C, C], f32r, kind="ExternalInput")
    wc_d = nc.dram_tensor("wc", [C, C], f32r, kind="ExternalInput")
    lamneg_d = nc.dram_tensor("lamneg", [P, NH], f32, kind="ExternalInput")
    out_d = nc.dram_tensor("out", [T, C], f32, kind="ExternalOutput")

    ln_bias = float(math.log(1.0 - LAMBDA_INIT))

    with tile.TileContext(nc) as tc:
        with (
            tc.tile_pool(name="const", bufs=1) as const,
            tc.tile_pool(name="ydata", bufs=8) as y_pool,
        ):
            ident = const.tile([P, P], f32, tag="ident")
            make_identity(nc, ident)
            lamneg = const.tile([P, NH], f32, tag="lamneg")
            nc.sync.dma_start(out=lamneg, in_=lamneg_d[:, :])
            den_store = const.tile([P, NH, 8], f32, tag="den")
            lnb = const.tile([P, 1], f32, tag="lnb")
            nc.vector.memset(lnb, ln_bias)

            y_tiles = [y_pool.tile([P, NH * P], f32, tag="y", name="yt") for _ in range(8)]
            mu_tiles = [y_pool.tile([P, NH], f32, tag="mu", name="mu") for _ in range(8)]
            var_tiles = [y_pool.tile([P, NH], f32, tag="var", name="var") for _ in range(8)]

            with (
                tc.tile_pool(name="xT", bufs=8) as xT_p,
                tc.tile_pool(name="vdata", bufs=8) as v_p,
            ):
                xT = [xT_p.tile([P, T], f32r, tag="xT", name="xT") for _ in range(8)]
                v_aug = [v_p.tile([P, NH, 132], f16, tag="v", name="vaug") for _ in range(8)]

                # ---------- Phase A: x transpose + V projection ----------
                with (
                    tc.tile_pool(name="xnat", bufs=3) as xnat_p,
                    tc.tile_pool(name="wv", bufs=8) as wv_p,
                    tc.tile_pool(name="psA", bufs=2, space="PSUM") as psA,
                    tc.tile_pool(name="psBv", bufs=4, space="PSUM") as psBv,
                ):
                    wv_sb = [wv_p.tile([P, C], f32r, tag="w", name="wsb") for _ in range(8)]
                    for c in range(8):
                        nc.gpsimd.dma_start(
                            out=wv_sb[c], in_=wv_d[128 * c : 128 * (c + 1), :]
                        )
                    for i in range(8):
                        xn = xnat_p.tile([P, C], f32, tag="xn")
                        nc.sync.dma_start(out=xn, in_=x_d[128 * i : 128 * (i + 1), :])
                        for jh in range(2):
                            pt = psA.tile([P, 512], f32, tag="psA")
                            for w in range(4):
                                j = 4 * jh + w
                                nc.tensor.transpose(
                                    out=pt[:, 128 * w : 128 * (w + 1)],
                                    in_=xn[:, 128 * j : 128 * (j + 1)],
                                    identity=ident,
                                )
                            for w in range(4):
                                j = 4 * jh + w
                                nc.any.tensor_copy(
                                    out=xT[j][:, 128 * i : 128 * (i + 1)],
                                    in_=pt[:, 128 * w : 128 * (w + 1)],
                                )
                    # V projection: out (T, vd); lhsT = xT tile, rhs = wv
                    for t in range(8):
                        for n in range(2):
                            ps = psBv.tile([P, 512], f32, tag="psBv")
                            for c in range(8):
                                nc.tensor.matmul(
                                    ps,
                                    lhsT=xT[c][:, 128 * t : 128 * (t + 1)],
                                    rhs=wv_sb[c][:, 512 * n : 512 * (n + 1)],
                                    start=(c == 0),
                                    stop=(c == 7),
                                )
                            nc.any.tensor_copy(
                                out=v_aug[t][:, 4 * n : 4 * (n + 1), 0:128],
                                in_=ps.rearrange("p (g d) -> p g d", g=4),
                            )
                        nc.vector.memset(v_aug[t][:, :, 128:129], 1.0)

                # ---------- Merged per-head projection + attention ----------
                with (
                    tc.tile_pool(name="wqk", bufs=20) as wqk_p,
                    tc.tile_pool(name="qk", bufs=3) as qk_p,
                    tc.tile_pool(name="pprob", bufs=4) as p_pool,
                    tc.tile_pool(name="smallc", bufs=16) as small,
                    tc.tile_pool(name="psB2", bufs=2, space="PSUM") as psB2,
                    tc.tile_pool(name="psS", bufs=4, space="PSUM") as psS,
                    tc.tile_pool(name="psY", bufs=2, space="PSUM") as psY,
                ):
                    def emit_wdma(h):
                        """DMA the column slices of Wq/Wk for head h."""
                        wq_t, wk_t = [], []
                        for c in range(8):
                            wt = wqk_p.tile([P, P], f32r, tag="wq", name="wqh")
                            nc.gpsimd.dma_start(
                                out=wt,
                                in_=wq_d[
                                    128 * c : 128 * (c + 1),
                                    128 * h : 128 * (h + 1),
                                ],
                            )
                            wq_t.append(wt)
                        for c in range(8):
                            wt = wqk_p.tile([P, P], f32r, tag="wk", name="wkh")
                            nc.gpsimd.dma_start(
                                out=wt,
                                in_=wk_d[
                                    128 * c : 128 * (c + 1),
                                    128 * h : 128 * (h + 1),
                                ],
                            )
                            wk_t.append(wt)
                        return wq_t, wk_t

                    def emit_proj(wt, dest):
                        """(head_dim 128, T) projection for one head."""
                        for n in range(2):
                            ps = psB2.tile([P, 512], f32, tag="psB2", name="pps")
                            for c in range(8):
                                nc.tensor.matmul(
                                    ps,
                                    lhsT=wt[c],
                                    rhs=xT[c][:, 512 * n : 512 * (n + 1)],
                                    start=(c == 0),
                                    stop=(c == 7),
                                )
                            nc.any.tensor_copy(
                                out=dest[:, 512 * n : 512 * (n + 1)], in_=ps
                            )

                    def emit_scores(h, qT, kT, pcs):
                        """Scores + exp + diag-mask, per k-tile, both streams
                        interleaved (concurrent PE row-groups)."""
                        for n in range(2):
                            for s in range(2):
                                pcs[(s, n)] = p_pool.tile(
                                    [P, 8, 512], f16, tag="p", name="pch"
                                )
                            for j in range(4 * n + 4):
                                qlo = min(128 * max(0, j - 4 * n), 256)
                                sp2 = [
                                    psS.tile([P, 512], f32, tag="psS", name="sp")
                                    for _ in range(2)
                                ]
                                for s in range(2):
                                    nc.tensor.matmul(
                                        sp2[s][:, qlo:512],
                                        lhsT=kT[
                                            64 * s : 64 * (s + 1),
                                            128 * j : 128 * (j + 1),
                                        ],
                                        rhs=qT[
                                            64 * s : 64 * (s + 1),
                                            512 * n + qlo : 512 * (n + 1),
                                        ],
                                        start=True,
                                        stop=True,
                                    )
                                t = j - 4 * n
                                for s in range(2):
                                    pch = pcs[(s, n)]
                                    nc.scalar.activation(
                                        out=pch[:, j, qlo:512],
                                        in_=sp2[s][:, qlo:512],
                                        func=Act.Exp,
                                        scale=0.125,
                                    )
                                    if 0 <= t <= 3:
                                        nc.gpsimd.affine_select(
                                            out=pch[:, j, 128 * t : 128 * (t + 1)],
                                            in_=pch[:, j, 128 * t : 128 * (t + 1)],
                                            compare_op=Alu.is_ge,
                                            fill=0.0,
                                            base=0,
                                            pattern=[[1, 128]],
                                            channel_multiplier=-1,
                                        )

                    def emit_pv(h, s, n, pcs):
                        """PV + stream-combine for q-tiles of chunk n."""
                        pch = pcs[(s, n)]
                        for i in range(4 * n, 4 * n + 4):
                            t = i % 4
                            yp = psY.tile([P, 129], f32, tag="psY", name="yp")
                            for j in range(i + 1):
                                nc.tensor.matmul(
                                    yp,
                                    lhsT=pch[:, j, 128 * t : 128 * (t + 1)],
                                    rhs=v_aug[j][:, h, 0:129],
                                    start=(j == 0),
                                    stop=(j == i),
                                )
                            ysl = y_tiles[i][:, 128 * h : 128 * (h + 1)]
                            if s == 0:
                                nc.scalar.copy(out=ysl, in_=yp[:, 0:128])
                                nc.vector.tensor_copy(
                                    out=den_store[:, h, i : i + 1],
                                    in_=yp[:, 128:129],
                                )
                            else:
                                r2 = small.tile([P, 1], f32, tag="r2", name="r2")
                                nc.vector.reciprocal(out=r2, in_=yp[:, 128:129])
                                gneg = small.tile([P, 1], f32, tag="gneg", name="gneg")
                                nc.vector.tensor_mul(
                                    out=gneg,
                                    in0=den_store[:, h, i : i + 1],
                                    in1=r2,
                                )
                                nc.vector.tensor_mul(
                                    out=gneg, in0=gneg, in1=lamneg[:, h : h + 1]
                                )
                                tmp = small.tile([P, P], f32, tag="tmp", name="tmp")
                                nc.scalar.activation(
                                    out=tmp,
                                    in_=yp[:, 0:128],
                                    func=Act.Copy,
                                    scale=gneg,
                                )
                                nc.vector.tensor_add(out=ysl, in0=ysl, in1=tmp)
                                # LN stats, overlapped with attention
                                bs = small.tile(
                                    [P, nc.vector.BN_STATS_DIM], f32,
                                    tag="bs", name="bs",
                                )
                                nc.vector.bn_stats(out=bs, in_=ysl)
                                mv = small.tile(
                                    [P, nc.vector.BN_AGGR_DIM], f32,
                                    tag="mv", name="mv",
                                )
                                nc.vector.bn_aggr(out=mv, in_=bs)
                                nc.vector.tensor_copy(
                                    out=mu_tiles[i][:, h : h + 1], in_=mv[:, 0:1]
                                )
                                nc.vector.tensor_copy(
                                    out=var_tiles[i][:, h : h + 1], in_=mv[:, 1:2]
                                )

                    # software pipeline: while head h's exp runs on ScalarE,
                    # PE projects head h+1
                    wts = emit_wdma(0)
                    qkts = None
                    pcs_prev = None
                    for h in range(NH):
                        qT = qk_p.tile([P, T], f32r, tag="q", name="qT")
                        kT = qk_p.tile([P, T], f32r, tag="k", name="kT")
                        emit_proj(wts[0], qT)
                        emit_proj(wts[1], kT)
                        if h + 1 < NH:
                            next_wts = emit_wdma(h + 1)
                        pcs = {}
                        emit_scores(h, qT, kT, pcs)
                        if pcs_prev is not None:
                            # PV of the previous head runs while this head's
                            # exp completes
                            for s in range(2):
                                for n in range(2):
                                    emit_pv(h - 1, s, n, pcs_prev)
                        pcs_prev = pcs
                        if h + 1 < NH:
                            wts = next_wts
                    for s in range(2):
                        for n in range(2):
                            emit_pv(NH - 1, s, n, pcs_prev)

            # ---------- Phase D/E/F: LN finalize, transpose, c_proj ----------
            with (
                tc.tile_pool(name="smalld", bufs=10) as sd,
                tc.tile_pool(name="ylnT", bufs=8) as ylnT_p,
                tc.tile_pool(name="wcp", bufs=8) as wc_p,
                tc.tile_pool(name="outp", bufs=3) as out_p,
                tc.tile_pool(name="psE", bufs=2, space="PSUM") as psE,
                tc.tile_pool(name="psF", bufs=4, space="PSUM") as psF,
            ):
                wc_sb = [wc_p.tile([P, C], f32r, tag="wc", name="wcsb") for _ in range(8)]
                for d in range(8):
                    nc.gpsimd.dma_start(
                        out=wc_sb[d], in_=wc_d[128 * d : 128 * (d + 1), :]
                    )

                # veps = var + eps*den1^2, batched Ln / Exp (one table set)
                veps_tiles = []
                for i in range(8):
                    d1 = den_store[:, :, i : i + 1].rearrange("p h one -> p (h one)")
                    veps = sd.tile([P, NH], f32, tag="veps")
                    nc.vector.tensor_mul(out=veps, in0=d1, in1=d1)
                    nc.vector.tensor_scalar(
                        out=veps, in0=veps, scalar1=LN_EPS, scalar2=None,
                        op0=Alu.mult,
                    )
                    nc.vector.tensor_add(out=veps, in0=veps, in1=var_tiles[i])
                    veps_tiles.append(veps)
                invstd_tiles = []
                for i in range(8):
                    lnv = sd.tile([P, NH], f32, tag="lnv")
                    nc.scalar.activation(out=lnv, in_=veps_tiles[i], func=Act.Ln)
                    invstd_tiles.append(lnv)
                for i in range(8):
                    nc.scalar.activation(
                        out=invstd_tiles[i], in_=invstd_tiles[i],
                        func=Act.Exp, scale=-0.5, bias=lnb,
                    )
                # per-i apply + transpose pipeline
                ylnT = [ylnT_p.tile([P, T], f32r, tag="ylnT", name="ylnT") for _ in range(8)]
                for i in range(8):
                    for h in range(NH):
                        nc.vector.tensor_scalar(
                            out=y_tiles[i][:, 128 * h : 128 * (h + 1)],
                            in0=y_tiles[i][:, 128 * h : 128 * (h + 1)],
                            scalar1=mu_tiles[i][:, h : h + 1],
                            scalar2=invstd_tiles[i][:, h : h + 1],
                            op0=Alu.subtract,
                            op1=Alu.mult,
                        )
                    for dh in range(2):
                        pt = psE.tile([P, 512], f32, tag="psE")
                        for w in range(4):
                            d = 4 * dh + w
                            nc.tensor.transpose(
                                out=pt[:, 128 * w : 128 * (w + 1)],
                                in_=y_tiles[i][:, 128 * d : 128 * (d + 1)],
                                identity=ident,
                            )
                        for w in range(4):
                            d = 4 * dh + w
                            nc.any.tensor_copy(
                                out=ylnT[d][:, 128 * i : 128 * (i + 1)],
                                in_=pt[:, 128 * w : 128 * (w + 1)],
                            )

                # c_proj
                for m in range(8):
                    osb = out_p.tile([P, C], f32, tag="osb")
                    for n in range(2):
                        ps = psF.tile([P, 512], f32, tag="psF")
                        for d in range(8):
                            nc.tensor.matmul(
                                ps,
                                lhsT=ylnT[d][:, 128 * m : 128 * (m + 1)],
                                rhs=wc_sb[d][:, 512 * n : 512 * (n + 1)],
                                start=(d == 0),
                                stop=(d == 7),
                            )
                        nc.any.tensor_copy(
                            out=osb[:, 512 * n : 512 * (n + 1)], in_=ps
                        )
                    nc.sync.dma_start(
                        out=out_d[128 * m : 128 * (m + 1), :], in_=osb
                    )

    bass._bass_rust.generate_event_semaphores(nc)
    return nc


_NC = None


def _get_program():
    global _NC
    if _NC is None:
        _NC = build_program()
    return _NC


def make_in_maps(inputs):
    """Host-side sharding: per-core input dicts."""
    x = np.ascontiguousarray(np.asarray(inputs["x"], dtype=np.float32))
    Wq1 = np.asarray(inputs["Wq1"], dtype=np.float32)
    Wq2 = np.asarray(inputs["Wq2"], dtype=np.float32)
    Wk1 = np.asarray(inputs["Wk1"], dtype=np.float32)
    Wk2 = np.asarray(inputs["Wk2"], dtype=np.float32)
    Wv = np.asarray(inputs["Wv"], dtype=np.float32)
    Wc = np.asarray(inputs["Wc"], dtype=np.float32)
    lq1 = np.asarray(inputs["lq1"], dtype=np.float32)
    lk1 = np.asarray(inputs["lk1"], dtype=np.float32)
    lq2 = np.asarray(inputs["lq2"], dtype=np.float32)
    lk2 = np.asarray(inputs["lk2"], dtype=np.float32)

    lam1 = np.exp(np.sum(lq1 * lk1, axis=-1))
    lam2 = np.exp(np.sum(lq2 * lk2, axis=-1))
    lam_full = (lam1 - lam2 + LAMBDA_INIT).astype(np.float32)  # (16,)

    in_maps = []
    for core in range(N_CORES):
        b, hg = core // 2, core % 2
        heads = np.arange(NH) + NH * hg  # global head idx
        wq = np.empty((C, C), np.float32)
        wk = np.empty((C, C), np.float32)
        wv = np.empty((C, C), np.float32)
        for h in range(NH):
            H = NH * hg + h
            wq[:, 128 * h : 128 * h + 64] = Wq1[:, HS * H : HS * (H + 1)]
            wq[:, 128 * h + 64 : 128 * (h + 1)] = Wq2[:, HS * H : HS * (H + 1)]
            wk[:, 128 * h : 128 * h + 64] = Wk1[:, HS * H : HS * (H + 1)]
            wk[:, 128 * h + 64 : 128 * (h + 1)] = Wk2[:, HS * H : HS * (H + 1)]
            wv[:, 128 * h : 128 * (h + 1)] = Wv[:, 128 * H : 128 * (H + 1)]
        wc = np.ascontiguousarray(Wc[1024 * hg : 1024 * (hg + 1), :])
        lamneg = np.broadcast_to(
            -lam_full[heads][None, :], (P, NH)
        ).astype(np.float32)
        in_maps.append(
            {
                "x": np.ascontiguousarray(x[b]),
                "wq": wq,
                "wk": wk,
                "wv": wv,
                "wc": wc,
                "lamneg": np.ascontiguousarray(lamneg),
            }
        )
    return in_maps


def run(inputs, trace=False, **kw):
    from concourse.bass_utils import run_bass_kernel_spmd

    nc = _get_program()
    in_maps = make_in_maps(inputs)
    res = run_bass_kernel_spmd(
        nc, in_maps, core_ids=list(range(N_CORES)), trace=trace, **kw
    )
    B = 4
    out = np.empty((B, T, C), np.float32)
    for b in range(B):
        out[b] = res.results[2 * b]["out"] + res.results[2 * b + 1]["out"]
    return out, res


def kernel(**inputs) -> np.ndarray:
    out, _ = run(inputs, trace=False)
    return out


# revision 16
# speedup vs baseline: 1.0380x; 1.0380x over previous
"""MultiHeadDiffAttention Trainium2 kernel (8 NeuronCores).

Sharding: batch (4) x head-group (2 groups of 8 heads) = 8 cores.
Each core computes a partial (T, C) c_proj output for its batch element
restricted to its 8 heads; the host sums the two head-group partials per
batch element.

Per-core pipeline (all matmuls on PE, fp32r for fp32 data, fp16 for the
attention probabilities / V):
  1. PE-transpose x[b] -> xT (C on partitions).
  2. Projections: Q1/Q2 and K1/K2 in (head_dim, T) layout (weights are
     host-interleaved so each 128-row chunk = one head's [q1|q2] dims);
     V in (T, vdim) layout with an appended ones column.
  3. Per head/stream: scores S^T(k,q) = K^T-tiles x Q^T (contract d=64),
     exp via ScalarE (fused 1/8 scale) -> fp16 P, causal mask via
     gpsimd memset/affine_select, then PV: P-tile^T x [V|1] accumulated
     over k-tiles gives Y and the softmax denominator in one matmul.
  4. Streams combined as z = Y1 - (lam*den1/den2) * Y2 (per-q scalars),
     which equals den1 * (a1 - lam*a2) @ V; LayerNorm is scale-invariant
     per row, so normalizing z with eps scaled by den1^2 reproduces the
     reference exactly.
  5. LN via bn_stats/bn_aggr + exp(-0.5*ln(var+eps*den1^2) + ln(1-li)).
  6. PE-transpose y_ln, c_proj vs host-sliced Wc rows -> partial out.
"""

import contextlib
import ctypes
import math
import sys
import types

import numpy as np

sys.path.insert(0, "/opt/trn_rl_repo")


def _install_ntff_hook():
    """Provide antenv.axon_hooks if the image lacks it (for trace=True)."""
    try:
        from antenv.axon_hooks import get_axon_ntff_profile_hook  # noqa: F401

        return
    except ImportError:
        pass

    so_path = "/opt/axon/libaxon_pjrt.so"

    def _make_hook():
        try:
            lib = ctypes.CDLL(so_path)
        except OSError:
            return None
        if not hasattr(lib, "axon_start_nrt_profile"):
            return None
        lib.axon_start_nrt_profile.argtypes = [
            ctypes.POINTER(ctypes.c_int64),
            ctypes.c_size_t,
        ]
        lib.axon_start_nrt_profile.restype = ctypes.c_int64
        lib.axon_stop_nrt_profile.argtypes = [ctypes.c_char_p]
        lib.axon_stop_nrt_profile.restype = ctypes.c_int64

        @contextlib.contextmanager
        def _hook(output_dir, device_ids):
            import jax

            jax.devices()
            if device_ids:
                ids = (ctypes.c_int64 * len(device_ids))(*device_ids)
                rc = lib.axon_start_nrt_profile(ids, len(device_ids))
            else:
                rc = lib.axon_start_nrt_profile(None, 0)
            if rc != 0:
                raise RuntimeError(f"axon_start_nrt_profile rc={rc}")
            try:
                yield
            finally:
                n = lib.axon_stop_nrt_profile(str(output_dir).encode())
                if n < 0:
                    raise RuntimeError(f"axon_stop_nrt_profile rc={n}")

        return _hook

    mod = types.ModuleType("antenv.axon_hooks")
    _the_hook = _make_hook()
    mod.get_axon_ntff_profile_hook = lambda: _the_hook
    sys.modules["antenv.axon_hooks"] = mod


_install_ntff_hook()

import concourse.bass as bass  # noqa: E402
import concourse.bass_utils as bass_utils_mod  # noqa: E402
import concourse.mybir as mybir  # noqa: E402
import concourse.tile as tile  # noqa: E402
from concourse.masks import make_identity  # noqa: E402


def _enable_ldw_opt():
    """walrus ships with --enable-ldw-opt=false hardcoded; the LDWEIGHTS
    optimization pass overlaps weight loads with in-flight matmuls, which
    matters a lot for this kernel (a fresh stationary operand per matmul
    in the PV stage). Rewrite the flag on the walrus command line."""
    orig = bass_utils_mod.run_command

    def patched(argv, **kwargs):
        argv = [
            "--enable-ldw-opt=true" if a == "--enable-ldw-opt=false" else a
            for a in argv
        ]
        return orig(argv, **kwargs)

    bass_utils_mod.run_command = patched


import os  # noqa: E402

if os.environ.get("BASS_LDW_OPT", "0") == "1":
    _enable_ldw_opt()

P = 128
T = 1024
C = 1024
NH = 8  # heads per core
HS = 64
LAMBDA_INIT = 0.8 - 0.6 * math.exp(-0.3 * (2 - 1))
LN_EPS = 1e-5
N_CORES = 8

f32 = mybir.dt.float32
f32r = mybir.dt.float32r
f16 = mybir.dt.float16
Alu = mybir.AluOpType
Act = mybir.ActivationFunctionType


def r(ap):
    return ap.bitcast(f32r)


def build_program():
    nc = bass.Bass()
    x_d = nc.dram_tensor("x", [T, C], f32, kind="ExternalInput")
    wq_d = nc.dram_tensor("wq", [C, C], f32r, kind="ExternalInput")
    wk_d = nc.dram_tensor("wk", [C, C], f32r, kind="ExternalInput")
    wv_d = nc.dram_tensor("wv", [C, C], f32r, kind="ExternalInput")
    wc_d = nc.dram_tensor("wc", [C, C], f32r, kind="ExternalInput")
    lamneg_d = nc.dram_tensor("lamneg", [P, NH], f32, kind="ExternalInput")
    out_d = nc.dram_tensor("out", [T, C], f32, kind="ExternalOutput")

    ln_bias = float(math.log(1.0 - LAMBDA_INIT))

    with tile.TileContext(nc) as tc:
        with (
            tc.tile_pool(name="const", bufs=1) as const,
            tc.tile_pool(name="ydata", bufs=8) as y_pool,
        ):
            ident = const.tile([P, P], f32, tag="ident")
            make_identity(nc, ident)
            lamneg = const.tile([P, NH], f32, tag="lamneg")
            nc.sync.dma_start(out=lamneg, in_=lamneg_d[:, :])
            den_store = const.tile([P, NH, 8], f32, tag="den")
            lnb = const.tile([P, 1], f32, tag="lnb")
            nc.vector.memset(lnb, ln_bias)

            y_tiles = [y_pool.tile([P, NH * P], f32, tag="y", name="yt") for _ in range(8)]
            mu_tiles = [y_pool.tile([P, NH], f32, tag="mu", name="mu") for _ in range(8)]
            var_tiles = [y_pool.tile([P, NH], f32, tag="var", name="var") for _ in range(8)]

            with (
                tc.tile_pool(name="xT", bufs=8) as xT_p,
                tc.tile_pool(name="vdata", bufs=8) as v_p,
            ):
                xT = [xT_p.tile([P, T], f32r, tag="xT", name="xT") for _ in range(8)]
                v_aug = [v_p.tile([P, NH, 132], f16, tag="v", name="vaug") for _ in range(8)]

                # ---------- Phase A: x transpose + V projection ----------
                with (
                    tc.tile_pool(name="xnat", bufs=3) as xnat_p,
                    tc.tile_pool(name="wv", bufs=8) as wv_p,
                    tc.tile_pool(name="psA", bufs=2, space="PSUM") as psA,
                    tc.tile_pool(name="psBv", bufs=4, space="PSUM") as psBv,
                ):
                    wv_sb = [wv_p.tile([P, C], f32r, tag="w", name="wsb") for _ in range(8)]
                    for c in range(8):
                        nc.gpsimd.dma_start(
                            out=wv_sb[c], in_=wv_d[128 * c : 128 * (c + 1), :]
                        )
                    for i in range(8):
                        xn = xnat_p.tile([P, C], f32, tag="xn")
                        nc.sync.dma_start(out=xn, in_=x_d[128 * i : 128 * (i + 1), :])
                        for jh in range(2):
                            pt = psA.tile([P, 512], f32, tag="psA")
                            for w in range(4):
                                j = 4 * jh + w
                                nc.tensor.transpose(
                                    out=pt[:, 128 * w : 128 * (w + 1)],
                                    in_=xn[:, 128 * j : 128 * (j + 1)],
                                    identity=ident,
                                )
                            for w in range(4):
                                j = 4 * jh + w
                                nc.any.tensor_copy(
                                    out=xT[j][:, 128 * i : 128 * (i + 1)],
                                    in_=pt[:, 128 * w : 128 * (w + 1)],
                                )
                    # V projection: out (T, vd); lhsT = xT tile, rhs = wv
                    for t in range(8):
                        for n in range(2):
                            ps = psBv.tile([P, 512], f32, tag="psBv")
                            for c in range(8):
                                nc.tensor.matmul(
                                    ps,
                                    lhsT=xT[c][:, 128 * t : 128 * (t + 1)],
                                    rhs=wv_sb[c][:, 512 * n : 512 * (n + 1)],
                                    start=(c == 0),
                                    stop=(c == 7),
                                )
                            nc.any.tensor_copy(
                                out=v_aug[t][:, 4 * n : 4 * (n + 1), 0:128],
                                in_=ps.rearrange("p (g d) -> p g d", g=4),
                            )
                        nc.vector.memset(v_aug[t][:, :, 128:129], 1.0)

                # ---------- Merged per-head projection + attention ----------
                with (
                    tc.tile_pool(name="wqk", bufs=3) as wqk_p,
                    tc.tile_pool(name="qk", bufs=3) as qk_p,
                    tc.tile_pool(name="pprob", bufs=4) as p_pool,
                    tc.tile_pool(name="smallc", bufs=16) as small,
                    tc.tile_pool(name="psB2", bufs=2, space="PSUM") as psB2,
                    tc.tile_pool(name="psS", bufs=4, space="PSUM") as psS,
                    tc.tile_pool(name="psY", bufs=2, space="PSUM") as psY,
                ):
                    def emit_wdma(h):
                        """One strided DMA per weight matrix for head h:
                        out[p, c, d] = w[128c + p, 128h + d]."""
                        tiles = []
                        for w_d, tag, nm in ((wq_d, "wq", "wqh"), (wk_d, "wk", "wkh")):
                            wt = wqk_p.tile([P, 8, P], f32r, tag=tag, name=nm)
                            src_ap = w_d.rearrange("(c p) d -> p c d", p=P)[
                                :, :, 128 * h : 128 * (h + 1)
                            ]
                            nc.sync.dma_start(out=wt, in_=src_ap)
                            tiles.append(wt)
                        return tiles

                    def emit_proj(wt, dest):
                        """(head_dim 128, T) projection for one head."""
                        for n in range(2):
                            ps = psB2.tile([P, 512], f32, tag="psB2", name="pps")
                            for c in range(8):
                                nc.tensor.matmul(
                                    ps,
                                    lhsT=wt[:, c, :],
                                    rhs=xT[c][:, 512 * n : 512 * (n + 1)],
                                    start=(c == 0),
                                    stop=(c == 7),
                                )
                            nc.vector.tensor_copy(
                                out=dest[:, 512 * n : 512 * (n + 1)], in_=ps
                            )

                    def emit_scores(h, qT, kT, pcs):
                        """Scores + exp + diag-mask, per k-tile, both streams
                        interleaved (concurrent PE row-groups)."""
                        for n in range(2):
                            for s in range(2):
                                pcs[(s, n)] = p_pool.tile(
                                    [P, 8, 512], f16, tag="p", name="pch"
                                )
                            for j in range(4 * n + 4):
                                qlo = min(128 * max(0, j - 4 * n), 256)
                                sp2 = [
                                    psS.tile([P, 512], f32, tag="psS", name="sp")
                                    for _ in range(2)
                                ]
                                for s in range(2):
                                    nc.tensor.matmul(
                                        sp2[s][:, qlo:512],
                                        lhsT=kT[
                                            64 * s : 64 * (s + 1),
                                            128 * j : 128 * (j + 1),
                                        ],
                                        rhs=qT[
                                            64 * s : 64 * (s + 1),
                                            512 * n + qlo : 512 * (n + 1),
                                        ],
                                        start=True,
                                        stop=True,
                                    )
                                t = j - 4 * n
                                for s in range(2):
                                    pch = pcs[(s, n)]
                                    nc.scalar.activation(
                                        out=pch[:, j, qlo:512],
                                        in_=sp2[s][:, qlo:512],
                                        func=Act.Exp,
                                        scale=0.125,
                                    )
                                    if 0 <= t <= 3:
                                        nc.gpsimd.affine_select(
                                            out=pch[:, j, 128 * t : 128 * (t + 1)],
                                            in_=pch[:, j, 128 * t : 128 * (t + 1)],
                                            compare_op=Alu.is_ge,
                                            fill=0.0,
                                            base=0,
                                            pattern=[[1, 128]],
                                            channel_multiplier=-1,
                                        )

                    def emit_pv(h, s, n, pcs):
                        """PV + stream-combine for q-tiles of chunk n."""
                        pch = pcs[(s, n)]
                        for i in range(4 * n, 4 * n + 4):
                            t = i % 4
                            yp = psY.tile([P, 129], f32, tag="psY", name="yp")
                            for j in range(i + 1):
                                nc.tensor.matmul(
                                    yp,
                                    lhsT=pch[:, j, 128 * t : 128 * (t + 1)],
                                    rhs=v_aug[j][:, h, 0:129],
                                    start=(j == 0),
                                    stop=(j == i),
                                )
                            ysl = y_tiles[i][:, 128 * h : 128 * (h + 1)]
                            if s == 0:
                                nc.vector.tensor_copy(out=ysl, in_=yp[:, 0:128])
                                nc.vector.tensor_copy(
                                    out=den_store[:, h, i : i + 1],
                                    in_=yp[:, 128:129],
                                )
                            else:
                                r2 = small.tile([P, 1], f32, tag="r2", name="r2")
                                nc.vector.reciprocal(out=r2, in_=yp[:, 128:129])
                                gneg = small.tile([P, 1], f32, tag="gneg", name="gneg")
                                nc.vector.tensor_mul(
                                    out=gneg,
                                    in0=den_store[:, h, i : i + 1],
                                    in1=r2,
                                )
                                nc.vector.tensor_mul(
                                    out=gneg, in0=gneg, in1=lamneg[:, h : h + 1]
                                )
                                tmp = small.tile([P, P], f32, tag="tmp", name="tmp")
                                nc.scalar.activation(
                                    out=tmp,
                                    in_=yp[:, 0:128],
                                    func=Act.Copy,
                                    scale=gneg,
                                )
                                nc.vector.tensor_add(out=ysl, in0=ysl, in1=tmp)
                                # LN stats, overlapped with attention
                                bs = small.tile(
                                    [P, nc.vector.BN_STATS_DIM], f32,
                                    tag="bs", name="bs",
                                )
                                nc.vector.bn_stats(out=bs, in_=ysl)
                                mv = small.tile(
                                    [P, nc.vector.BN_AGGR_DIM], f32,
                                    tag="mv", name="mv",
                                )
                                nc.vector.bn_aggr(out=mv, in_=bs)
                                nc.vector.tensor_copy(
                                    out=mu_tiles[i][:, h : h + 1], in_=mv[:, 0:1]
                                )
                                nc.vector.tensor_copy(
                                    out=var_tiles[i][:, h : h + 1], in_=mv[:, 1:2]
                                )

                    # software pipeline: while head h's exp runs on ScalarE,
                    # PE projects head h+1
                    wts = emit_wdma(0)
                    qkts = None
                    pcs_prev = None
                    for h in range(NH):
                        qT = qk_p.tile([P, T], f32r, tag="q", name="qT")
                        kT = qk_p.tile([P, T], f32r, tag="k", name="kT")
                        emit_proj(wts[0], qT)
                        emit_proj(wts[1], kT)
                        if h + 1 < NH:
                            next_wts = emit_wdma(h + 1)
                        pcs = {}
                        emit_scores(h, qT, kT, pcs)
                        if pcs_prev is not None:
                            # PV of the previous head runs while this head's
                            # exp completes
                            for s in range(2):
                                for n in range(2):
                                    emit_pv(h - 1, s, n, pcs_prev)
                        pcs_prev = pcs
                        if h + 1 < NH:
                            wts = next_wts
                    for s in range(2):
                        for n in range(2):
                            emit_pv(NH - 1, s, n, pcs_prev)

            # ---------- Phase D/E/F: LN finalize, transpose, c_proj ----------
            with (
                tc.tile_pool(name="smalld", bufs=10) as sd,
                tc.tile_pool(name="ylnT", bufs=8) as ylnT_p,
                tc.tile_pool(name="wcp", bufs=8) as wc_p,
                tc.tile_pool(name="outp", bufs=3) as out_p,
                tc.tile_pool(name="psE", bufs=2, space="PSUM") as psE,
                tc.tile_pool(name="psF", bufs=4, space="PSUM") as psF,
            ):
                wc_sb = [wc_p.tile([P, C], f32r, tag="wc", name="wcsb") for _ in range(8)]
                for d in range(8):
                    nc.gpsimd.dma_start(
                        out=wc_sb[d], in_=wc_d[128 * d : 128 * (d + 1), :]
                    )

                # veps = var + eps*den1^2, batched Ln / Exp (one table set)
                veps_tiles = []
                for i in range(8):
                    d1 = den_store[:, :, i : i + 1].rearrange("p h one -> p (h one)")
                    veps = sd.tile([P, NH], f32, tag="veps")
                    nc.vector.tensor_mul(out=veps, in0=d1, in1=d1)
                    nc.vector.tensor_scalar(
                        out=veps, in0=veps, scalar1=LN_EPS, scalar2=None,
                        op0=Alu.mult,
                    )
                    nc.vector.tensor_add(out=veps, in0=veps, in1=var_tiles[i])
                    veps_tiles.append(veps)
                invstd_tiles = []
                for i in range(8):
                    lnv = sd.tile([P, NH], f32, tag="lnv")
                    nc.scalar.activation(out=lnv, in_=veps_tiles[i], func=Act.Ln)
                    invstd_tiles.append(lnv)
                for i in range(8):
                    nc.scalar.activation(
                        out=invstd_tiles[i], in_=invstd_tiles[i],
                        func=Act.Exp, scale=-0.5, bias=lnb,
                    )
                # per-i apply + transpose pipeline
                ylnT = [ylnT_p.tile([P, T], f32r, tag="ylnT", name="ylnT") for _ in range(8)]
                for i in range(8):
                    for h in range(NH):
                        nc.vector.tensor_scalar(
                            out=y_tiles[i][:, 128 * h : 128 * (h + 1)],
                            in0=y_tiles[i][:, 128 * h : 128 * (h + 1)],
                            scalar1=mu_tiles[i][:, h : h + 1],
                            scalar2=invstd_tiles[i][:, h : h + 1],
                            op0=Alu.subtract,
                            op1=Alu.mult,
                        )
                    for dh in range(2):
                        pt = psE.tile([P, 512], f32, tag="psE")
                        for w in range(4):
                            d = 4 * dh + w
                            nc.tensor.transpose(
                                out=pt[:, 128 * w : 128 * (w + 1)],
                                in_=y_tiles[i][:, 128 * d : 128 * (d + 1)],
                                identity=ident,
                            )
                        for w in range(4):
                            d = 4 * dh + w
                            nc.any.tensor_copy(
                                out=ylnT[d][:, 128 * i : 128 * (i + 1)],
                                in_=pt[:, 128 * w : 128 * (w + 1)],
                            )

                # c_proj
                for m in range(8):
                    osb = out_p.tile([P, C], f32, tag="osb")
                    for n in range(2):
                        ps = psF.tile([P, 512], f32, tag="psF")
                        for d in range(8):
                            nc.tensor.matmul(
                                ps,
                                lhsT=ylnT[d][:, 128 * m : 128 * (m + 1)],
                                rhs=wc_sb[d][:, 512 * n : 512 * (n + 1)],
                                start=(d == 0),
                                stop=(d == 7),
                            )
                        nc.any.tensor_copy(
                            out=osb[:, 512 * n : 512 * (n + 1)], in_=ps
                        )
                    nc.sync.dma_start(
                        out=out_d[128 * m : 128 * (m + 1), :], in_=osb
                    )

    bass._bass_rust.generate_event_semaphores(nc)
    return nc


_NC = None


def _get_program():
    global _NC
    if _NC is None:
        _NC = build_program()
    return _NC


def make_in_maps(inputs):
    """Host-side sharding: per-core input dicts."""
    x = np.ascontiguousarray(np.asarray(inputs["x"], dtype=np.float32))
    Wq1 = np.asarray(inputs["Wq1"], dtype=np.float32)
    Wq2 = np.asarray(inputs["Wq2"], dtype=np.float32)
    Wk1 = np.asarray(inputs["Wk1"], dtype=np.float32)
    Wk2 = np.asarray(inputs["Wk2"], dtype=np.float32)
    Wv = np.asarray(inputs["Wv"], dtype=np.float32)
    Wc = np.asarray(inputs["Wc"], dtype=np.float32)
    lq1 = np.asarray(inputs["lq1"], dtype=np.float32)
    lk1 = np.asarray(inputs["lk1"], dtype=np.float32)
    lq2 = np.asarray(inputs["lq2"], dtype=np.float32)
    lk2 = np.asarray(inputs["lk2"], dtype=np.float32)

    lam1 = np.exp(np.sum(lq1 * lk1, axis=-1))
    lam2 = np.exp(np.sum(lq2 * lk2, axis=-1))
    lam_full = (lam1 - lam2 + LAMBDA_INIT).astype(np.float32)  # (16,)

    in_maps = []
    for core in range(N_CORES):
        b, hg = core // 2, core % 2
        heads = np.arange(NH) + NH * hg  # global head idx
        wq = np.empty((C, C), np.float32)
        wk = np.empty((C, C), np.float32)
        wv = np.empty((C, C), np.float32)
        for h in range(NH):
            H = NH * hg + h
            wq[:, 128 * h : 128 * h + 64] = Wq1[:, HS * H : HS * (H + 1)]
            wq[:, 128 * h + 64 : 128 * (h + 1)] = Wq2[:, HS * H : HS * (H + 1)]
            wk[:, 128 * h : 128 * h + 64] = Wk1[:, HS * H : HS * (H + 1)]
            wk[:, 128 * h + 64 : 128 * (h + 1)] = Wk2[:, HS * H : HS * (H + 1)]
            wv[:, 128 * h : 128 * (h + 1)] = Wv[:, 128 * H : 128 * (H + 1)]
        wc = np.ascontiguousarray(Wc[1024 * hg : 1024 * (hg + 1), :])
        lamneg = np.broadcast_to(
            -lam_full[heads][None, :], (P, NH)
        ).astype(np.float32)
        in_maps.append(
            {
                "x": np.ascontiguousarray(x[b]),
                "wq": wq,
                "wk": wk,
                "wv": wv,
                "wc": wc,
                "lamneg": np.ascontiguousarray(lamneg),
            }
        )
    return in_maps


def run(inputs, trace=False, **kw):
    from concourse.bass_utils import run_bass_kernel_spmd

    nc = _get_program()
    in_maps = make_in_maps(inputs)
    res = run_bass_kernel_spmd(
        nc, in_maps, core_ids=list(range(N_CORES)), trace=trace, **kw
    )
    B = 4
    out = np.empty((B, T, C), np.float32)
    for b in range(B):
        out[b] = res.results[2 * b]["out"] + res.results[2 * b + 1]["out"]
    return out, res


def kernel(**inputs) -> np.ndarray:
    out, _ = run(inputs, trace=False)
    return out


# revision 20
# speedup vs baseline: 1.2003x; 1.1564x over previous
"""MultiHeadDiffAttention Trainium2 kernel (8 NeuronCores).

Sharding: batch (4) x head-group (2 groups of 8 heads) = 8 cores.
Each core computes a partial (T, C) c_proj output for its batch element
restricted to its 8 heads; the host sums the two head-group partials per
batch element.

Per-core pipeline (all matmuls on PE, fp32r for fp32 data, fp16 for the
attention probabilities / V):
  1. PE-transpose x[b] -> xT (C on partitions).
  2. Projections: Q1/Q2 and K1/K2 in (head_dim, T) layout (weights are
     host-interleaved so each 128-row chunk = one head's [q1|q2] dims);
     V in (T, vdim) layout with an appended ones column.
  3. Per head/stream: scores S^T(k,q) = K^T-tiles x Q^T (contract d=64),
     exp via ScalarE (fused 1/8 scale) -> fp16 P, causal mask via
     gpsimd memset/affine_select, then PV: P-tile^T x [V|1] accumulated
     over k-tiles gives Y and the softmax denominator in one matmul.
  4. Streams combined as z = Y1 - (lam*den1/den2) * Y2 (per-q scalars),
     which equals den1 * (a1 - lam*a2) @ V; LayerNorm is scale-invariant
     per row, so normalizing z with eps scaled by den1^2 reproduces the
     reference exactly.
  5. LN via bn_stats/bn_aggr + exp(-0.5*ln(var+eps*den1^2) + ln(1-li)).
  6. PE-transpose y_ln, c_proj vs host-sliced Wc rows -> partial out.
"""

import contextlib
import ctypes
import math
import sys
import types

import numpy as np

sys.path.insert(0, "/opt/trn_rl_repo")


def _install_ntff_hook():
    """Provide antenv.axon_hooks if the image lacks it (for trace=True)."""
    try:
        from antenv.axon_hooks import get_axon_ntff_profile_hook  # noqa: F401

        return
    except ImportError:
        pass

    so_path = "/opt/axon/libaxon_pjrt.so"

    def _make_hook():
        try:
            lib = ctypes.CDLL(so_path)
        except OSError:
            return None
        if not hasattr(lib, "axon_start_nrt_profile"):
            return None
        lib.axon_start_nrt_profile.argtypes = [
            ctypes.POINTER(ctypes.c_int64),
            ctypes.c_size_t,
        ]
        lib.axon_start_nrt_profile.restype = ctypes.c_int64
        lib.axon_stop_nrt_profile.argtypes = [ctypes.c_char_p]
        lib.axon_stop_nrt_profile.restype = ctypes.c_int64

        @contextlib.contextmanager
        def _hook(output_dir, device_ids):
            import jax

            jax.devices()
            if device_ids:
                ids = (ctypes.c_int64 * len(device_ids))(*device_ids)
                rc = lib.axon_start_nrt_profile(ids, len(device_ids))
            else:
                rc = lib.axon_start_nrt_profile(None, 0)
            if rc != 0:
                raise RuntimeError(f"axon_start_nrt_profile rc={rc}")
            try:
                yield
            finally:
                n = lib.axon_stop_nrt_profile(str(output_dir).encode())
                if n < 0:
                    raise RuntimeError(f"axon_stop_nrt_profile rc={n}")

        return _hook

    mod = types.ModuleType("antenv.axon_hooks")
    _the_hook = _make_hook()
    mod.get_axon_ntff_profile_hook = lambda: _the_hook
    sys.modules["antenv.axon_hooks"] = mod


_install_ntff_hook()

import concourse.bass as bass  # noqa: E402
import concourse.bass_utils as bass_utils_mod  # noqa: E402
import concourse.mybir as mybir  # noqa: E402
import concourse.tile as tile  # noqa: E402
from concourse.masks import make_identity  # noqa: E402


def _enable_ldw_opt():
    """walrus ships with --enable-ldw-opt=false hardcoded; the LDWEIGHTS
    optimization pass overlaps weight loads with in-flight matmuls, which
    matters a lot for this kernel (a fresh stationary operand per matmul
    in the PV stage). Rewrite the flag on the walrus command line."""
    orig = bass_utils_mod.run_command

    def patched(argv, **kwargs):
        argv = [
            "--enable-ldw-opt=true" if a == "--enable-ldw-opt=false" else a
            for a in argv
        ]
        return orig(argv, **kwargs)

    bass_utils_mod.run_command = patched


import os  # noqa: E402

if os.environ.get("BASS_LDW_OPT", "0") == "1":
    _enable_ldw_opt()

P = 128
T = 1024
C = 1024
NH = 8  # heads per core
HS = 64
LAMBDA_INIT = 0.8 - 0.6 * math.exp(-0.3 * (2 - 1))
LN_EPS = 1e-5
N_CORES = 8

f32 = mybir.dt.float32
f32r = mybir.dt.float32r
f16 = mybir.dt.float16
Alu = mybir.AluOpType
Act = mybir.ActivationFunctionType


def r(ap):
    return ap.bitcast(f32r)


def build_program():
    nc = bass.Bass()
    x_d = nc.dram_tensor("x", [T, C], f32, kind="ExternalInput")
    wq_d = nc.dram_tensor("wq", [C, C], f32r, kind="ExternalInput")
    wk_d = nc.dram_tensor("wk", [C, C], f32r, kind="ExternalInput")
    wv_d = nc.dram_tensor("wv", [C, C], f32r, kind="ExternalInput")
    wc_d = nc.dram_tensor("wc", [C, C], f32r, kind="ExternalInput")
    lamneg_d = nc.dram_tensor("lamneg", [P, NH], f32, kind="ExternalInput")
    out_d = nc.dram_tensor("out", [T, C], f32, kind="ExternalOutput")

    ln_bias = float(math.log(1.0 - LAMBDA_INIT))

    with tile.TileContext(nc) as tc:
        with (
            tc.tile_pool(name="const", bufs=1) as const,
            tc.tile_pool(name="ydata", bufs=8) as y_pool,
        ):
            ident = const.tile([P, P], f32, tag="ident")
            make_identity(nc, ident)
            lamneg = const.tile([P, NH], f32, tag="lamneg")
            nc.sync.dma_start(out=lamneg, in_=lamneg_d[:, :])
            den_store = const.tile([P, NH, 8], f32, tag="den")
            lnb = const.tile([P, 1], f32, tag="lnb")
            nc.vector.memset(lnb, ln_bias)

            y_tiles = [y_pool.tile([P, NH * P], f32, tag="y", name="yt") for _ in range(8)]
            mu_tiles = [y_pool.tile([P, NH], f32, tag="mu", name="mu") for _ in range(8)]
            var_tiles = [y_pool.tile([P, NH], f32, tag="var", name="var") for _ in range(8)]

            with (
                tc.tile_pool(name="xT", bufs=8) as xT_p,
                tc.tile_pool(name="vdata", bufs=8) as v_p,
            ):
                xT = [xT_p.tile([P, T], f32r, tag="xT", name="xT") for _ in range(8)]
                v_aug = [v_p.tile([P, NH, 132], f16, tag="v", name="vaug") for _ in range(8)]

                # ---------- Phase A: x transpose + V projection ----------
                wv_ctx = tc.tile_pool(name="wv", bufs=8)
                wv_p = wv_ctx.__enter__()
                with (
                    tc.tile_pool(name="xnat", bufs=3) as xnat_p,
                    tc.tile_pool(name="psA", bufs=2, space="PSUM") as psA,
                    tc.tile_pool(name="psBv", bufs=4, space="PSUM") as psBv,
                ):
                    wv_sb = [wv_p.tile([P, C], f32r, tag="w", name="wsb") for _ in range(8)]
                    for c in range(8):
                        nc.gpsimd.dma_start(
                            out=wv_sb[c], in_=wv_d[128 * c : 128 * (c + 1), :]
                        )
                    for i in range(8):
                        xn = xnat_p.tile([P, C], f32, tag="xn")
                        nc.sync.dma_start(out=xn, in_=x_d[128 * i : 128 * (i + 1), :])
                        for jh in range(2):
                            pt = psA.tile([P, 512], f32, tag="psA")
                            for w in range(4):
                                j = 4 * jh + w
                                nc.tensor.transpose(
                                    out=pt[:, 128 * w : 128 * (w + 1)],
                                    in_=xn[:, 128 * j : 128 * (j + 1)],
                                    identity=ident,
                                )
                            for w in range(4):
                                j = 4 * jh + w
                                nc.any.tensor_copy(
                                    out=xT[j][:, 128 * i : 128 * (i + 1)],
                                    in_=pt[:, 128 * w : 128 * (w + 1)],
                                )
                    # V projection for t-tiles 0..3; 4..7 are emitted
                    # interleaved with head 0's scores to keep PE dense
                    for t in range(4):
                        for n in range(2):
                            ps = psBv.tile([P, 512], f32, tag="psBv")
                            for c in range(8):
                                nc.tensor.matmul(
                                    ps,
                                    lhsT=xT[c][:, 128 * t : 128 * (t + 1)],
                                    rhs=wv_sb[c][:, 512 * n : 512 * (n + 1)],
                                    start=(c == 0),
                                    stop=(c == 7),
                                )
                            nc.any.tensor_copy(
                                out=v_aug[t][:, 4 * n : 4 * (n + 1), 0:128],
                                in_=ps.rearrange("p (g d) -> p g d", g=4),
                            )
                        nc.vector.memset(v_aug[t][:, :, 128:129], 1.0)

                # ---------- Merged per-head projection + attention ----------
                with (
                    tc.tile_pool(name="wqk", bufs=2) as wqk_p,
                    tc.tile_pool(name="qk", bufs=2) as qk_p,
                    tc.tile_pool(name="pprob", bufs=8) as p_pool,
                    tc.tile_pool(name="smallc", bufs=16) as small,
                    tc.tile_pool(name="psB2", bufs=2, space="PSUM") as psB2,
                    tc.tile_pool(name="psS", bufs=4, space="PSUM") as psS,
                    tc.tile_pool(name="psY", bufs=2, space="PSUM") as psY,
                ):
                    def emit_wdma(h):
                        """One strided DMA per weight matrix for head h:
                        out[p, c, d] = w[128c + p, 128h + d]."""
                        tiles = []
                        for w_d, tag, nm in ((wq_d, "wq", "wqh"), (wk_d, "wk", "wkh")):
                            wt = wqk_p.tile([P, 8, P], f32r, tag=tag, name=nm)
                            src_ap = w_d.rearrange("(c p) d -> p c d", p=P)[
                                :, :, 128 * h : 128 * (h + 1)
                            ]
                            nc.sync.dma_start(out=wt, in_=src_ap)
                            tiles.append(wt)
                        return tiles

                    def emit_proj(wt, dest):
                        """(head_dim 128, T) projection for one head."""
                        for n in range(2):
                            ps = psB2.tile([P, 512], f32, tag="psB2", name="pps")
                            for c in range(8):
                                nc.tensor.matmul(
                                    ps,
                                    lhsT=wt[:, c, :],
                                    rhs=xT[c][:, 512 * n : 512 * (n + 1)],
                                    start=(c == 0),
                                    stop=(c == 7),
                                )
                            nc.vector.tensor_copy(
                                out=dest[:, 512 * n : 512 * (n + 1)], in_=ps
                            )

                    def score_unit(h, qT, kT, pcs, n, j):
                        """One k-tile of scores: both streams' matmuls in
                        concurrent PE row-groups, then exp + diag mask."""
                        qlo = min(128 * max(0, j - 4 * n), 256)
                        sp2 = [
                            psS.tile([P, 512], f32, tag="psS", name="sp")
                            for _ in range(2)
                        ]
                        for s in range(2):
                            nc.tensor.matmul(
                                sp2[s][:, qlo:512],
                                lhsT=kT[
                                    64 * s : 64 * (s + 1),
                                    128 * j : 128 * (j + 1),
                                ],
                                rhs=qT[
                                    64 * s : 64 * (s + 1),
                                    512 * n + qlo : 512 * (n + 1),
                                ],
                                start=True,
                                stop=True,
                            )
                        t = j - 4 * n
                        for s in range(2):
                            pch = pcs[(s, n)]
                            nc.scalar.activation(
                                out=pch[:, j, qlo:512],
                                in_=sp2[s][:, qlo:512],
                                func=Act.Exp,
                                scale=0.125,
                            )
                            if 0 <= t <= 3:
                                nc.gpsimd.affine_select(
                                    out=pch[:, j, 128 * t : 128 * (t + 1)],
                                    in_=pch[:, j, 128 * t : 128 * (t + 1)],
                                    compare_op=Alu.is_ge,
                                    fill=0.0,
                                    base=0,
                                    pattern=[[1, 128]],
                                    channel_multiplier=-1,
                                )

                    def pv_unit(h, s, i, pcs):
                        """PV + stream-combine for one q-tile."""
                        n, t = i // 4, i % 4
                        pch = pcs[(s, n)]
                        yp = psY.tile([P, 129], f32, tag="psY", name="yp")
                        for j in range(i + 1):
                            nc.tensor.matmul(
                                yp,
                                lhsT=pch[:, j, 128 * t : 128 * (t + 1)],
                                rhs=v_aug[j][:, h, 0:129],
                                start=(j == 0),
                                stop=(j == i),
                            )
                        ysl = y_tiles[i][:, 128 * h : 128 * (h + 1)]
                        if s == 0:
                            nc.vector.tensor_copy(out=ysl, in_=yp[:, 0:128])
                            nc.vector.tensor_copy(
                                out=den_store[:, h, i : i + 1],
                                in_=yp[:, 128:129],
                            )
                        else:
                            r2 = small.tile([P, 1], f32, tag="r2", name="r2")
                            nc.vector.reciprocal(out=r2, in_=yp[:, 128:129])
                            gneg = small.tile([P, 1], f32, tag="gneg", name="gneg")
                            nc.vector.tensor_mul(
                                out=gneg,
                                in0=den_store[:, h, i : i + 1],
                                in1=r2,
                            )
                            nc.vector.tensor_mul(
                                out=gneg, in0=gneg, in1=lamneg[:, h : h + 1]
                            )
                            tmp = small.tile([P, P], f32, tag="tmp", name="tmp")
                            nc.scalar.activation(
                                out=tmp,
                                in_=yp[:, 0:128],
                                func=Act.Copy,
                                scale=gneg,
                            )
                            nc.vector.tensor_add(out=ysl, in0=ysl, in1=tmp)
                            bs = small.tile(
                                [P, nc.vector.BN_STATS_DIM], f32,
                                tag="bs", name="bs",
                            )
                            nc.vector.bn_stats(out=bs, in_=ysl)
                            mv = small.tile(
                                [P, nc.vector.BN_AGGR_DIM], f32,
                                tag="mv", name="mv",
                            )
                            nc.vector.bn_aggr(out=mv, in_=bs)
                            nc.vector.tensor_copy(
                                out=mu_tiles[i][:, h : h + 1], in_=mv[:, 0:1]
                            )
                            nc.vector.tensor_copy(
                                out=var_tiles[i][:, h : h + 1], in_=mv[:, 1:2]
                            )

                    def vproj_unit(t, n):
                        """Deferred V-projection tile (t in 4..7)."""
                        ps = psB2.tile([P, 512], f32, tag="psB2", name="pps")
                        for c in range(8):
                            nc.tensor.matmul(
                                ps,
                                lhsT=xT[c][:, 128 * t : 128 * (t + 1)],
                                rhs=wv_sb[c][:, 512 * n : 512 * (n + 1)],
                                start=(c == 0),
                                stop=(c == 7),
                            )
                        nc.any.tensor_copy(
                            out=v_aug[t][:, 4 * n : 4 * (n + 1), 0:128],
                            in_=ps.rearrange("p (g d) -> p g d", g=4),
                        )
                        if n == 1:
                            nc.vector.memset(v_aug[t][:, :, 128:129], 1.0)

                    # software pipeline: PV matmuls of head h-1 (or the
                    # deferred V-projection for h=0) are interleaved between
                    # head h's score units so PE never stalls on exp
                    wts = emit_wdma(0)
                    pcs_prev = None
                    for h in range(NH):
                        qT = qk_p.tile([P, T], f32r, tag="q", name="qT")
                        kT = qk_p.tile([P, T], f32r, tag="k", name="kT")
                        emit_proj(wts[0], qT)
                        emit_proj(wts[1], kT)
                        if h + 1 < NH:
                            next_wts = emit_wdma(h + 1)
                        pcs = {
                            (s, n): p_pool.tile(
                                [P, 4 * n + 4, 512], f16,
                                tag=f"p{n}", name="pch", bufs=4,
                            )
                            for s in range(2)
                            for n in range(2)
                        }
                        if h == 0:
                            backlog = [("v", t, n) for t in range(4, 8) for n in range(2)]
                        else:
                            backlog = [("pv", s, i) for i in range(8) for s in range(2)]
                        sunits = [(n, j) for n in range(2) for j in range(4 * n + 4)]
                        done = 0
                        for idx, (n, j) in enumerate(sunits):
                            score_unit(h, qT, kT, pcs, n, j)
                            while done < len(backlog) and (idx + 1) * len(
                                backlog
                            ) >= (done + 1) * len(sunits):
                                u = backlog[done]
                                done += 1
                                if u[0] == "v":
                                    vproj_unit(u[1], u[2])
                                else:
                                    pv_unit(h - 1, u[1], u[2], pcs_prev)
                        while done < len(backlog):
                            u = backlog[done]
                            done += 1
                            if u[0] == "v":
                                vproj_unit(u[1], u[2])
                            else:
                                pv_unit(h - 1, u[1], u[2], pcs_prev)
                        pcs_prev = pcs
                        if h + 1 < NH:
                            wts = next_wts
                    for i in range(8):
                        for s in range(2):
                            pv_unit(NH - 1, s, i, pcs_prev)
                wv_ctx.__exit__(None, None, None)

            # ---------- Phase D/E/F: LN finalize, transpose, c_proj ----------
            with (
                tc.tile_pool(name="smalld", bufs=10) as sd,
                tc.tile_pool(name="ylnT", bufs=8) as ylnT_p,
                tc.tile_pool(name="wcp", bufs=8) as wc_p,
                tc.tile_pool(name="outp", bufs=3) as out_p,
                tc.tile_pool(name="psE", bufs=2, space="PSUM") as psE,
                tc.tile_pool(name="psF", bufs=4, space="PSUM") as psF,
            ):
                wc_sb = [wc_p.tile([P, C], f32r, tag="wc", name="wcsb") for _ in range(8)]
                for d in range(8):
                    nc.gpsimd.dma_start(
                        out=wc_sb[d], in_=wc_d[128 * d : 128 * (d + 1), :]
                    )

                # veps = var + eps*den1^2, batched Ln / Exp (one table set)
                veps_tiles = []
                for i in range(8):
                    d1 = den_store[:, :, i : i + 1].rearrange("p h one -> p (h one)")
                    veps = sd.tile([P, NH], f32, tag="veps")
                    nc.vector.tensor_mul(out=veps, in0=d1, in1=d1)
                    nc.vector.tensor_scalar(
                        out=veps, in0=veps, scalar1=LN_EPS, scalar2=None,
                        op0=Alu.mult,
                    )
                    nc.vector.tensor_add(out=veps, in0=veps, in1=var_tiles[i])
                    veps_tiles.append(veps)
                invstd_tiles = []
                for i in range(8):
                    lnv = sd.tile([P, NH], f32, tag="lnv")
                    nc.scalar.activation(out=lnv, in_=veps_tiles[i], func=Act.Ln)
                    invstd_tiles.append(lnv)
                for i in range(8):
                    nc.scalar.activation(
                        out=invstd_tiles[i], in_=invstd_tiles[i],
                        func=Act.Exp, scale=-0.5, bias=lnb,
                    )
                # per-i apply + transpose pipeline
                ylnT = [ylnT_p.tile([P, T], f32r, tag="ylnT", name="ylnT") for _ in range(8)]
                for i in range(8):
                    for h in range(NH):
                        nc.vector.tensor_scalar(
                            out=y_tiles[i][:, 128 * h : 128 * (h + 1)],
                            in0=y_tiles[i][:, 128 * h : 128 * (h + 1)],
                            scalar1=mu_tiles[i][:, h : h + 1],
                            scalar2=invstd_tiles[i][:, h : h + 1],
                            op0=Alu.subtract,
                            op1=Alu.mult,
                        )
                    for dh in range(2):
                        pt = psE.tile([P, 512], f32, tag="psE")
                        for w in range(4):
                            d = 4 * dh + w
                            nc.tensor.transpose(
                                out=pt[:, 128 * w : 128 * (w + 1)],
                                in_=y_tiles[i][:, 128 * d : 128 * (d + 1)],
                                identity=ident,
                            )
                        for w in range(4):
                            d = 4 * dh + w
                            nc.any.tensor_copy(
                                out=ylnT[d][:, 128 * i : 128 * (i + 1)],
                                in_=pt[:, 128 * w : 128 * (w + 1)],
                            )

                # c_proj
                for m in range(8):
                    osb = out_p.tile([P, C], f32, tag="osb")
                    for n in range(2):
                        ps = psF.tile([P, 512], f32, tag="psF")
                        for d in range(8):
                            nc.tensor.matmul(
                                ps,
                                lhsT=ylnT[d][:, 128 * m : 128 * (m + 1)],
                                rhs=wc_sb[d][:, 512 * n : 512 * (n + 1)],
                                start=(d == 0),
                                stop=(d == 7),
                            )
                        nc.any.tensor_copy(
                            out=osb[:, 512 * n : 512 * (n + 1)], in_=ps
                        )
                    nc.sync.dma_start(
                        out=out_d[128 * m : 128 * (m + 1), :], in_=osb
                    )

    bass._bass_rust.generate_event_semaphores(nc)
    return nc


_NC = None


def _get_program():
    global _NC
    if _NC is None:
        _NC = build_program()
    return _NC


def make_in_maps(inputs):
    """Host-side sharding: per-core input dicts."""
    x = np.ascontiguousarray(np.asarray(inputs["x"], dtype=np.float32))
    Wq1 = np.asarray(inputs["Wq1"], dtype=np.float32)
    Wq2 = np.asarray(inputs["Wq2"], dtype=np.float32)
    Wk1 = np.asarray(inputs["Wk1"], dtype=np.float32)
    Wk2 = np.asarray(inputs["Wk2"], dtype=np.float32)
    Wv = np.asarray(inputs["Wv"], dtype=np.float32)
    Wc = np.asarray(inputs["Wc"], dtype=np.float32)
    lq1 = np.asarray(inputs["lq1"], dtype=np.float32)
    lk1 = np.asarray(inputs["lk1"], dtype=np.float32)
    lq2 = np.asarray(inputs["lq2"], dtype=np.float32)
    lk2 = np.asarray(inputs["lk2"], dtype=np.float32)

    lam1 = np.exp(np.sum(lq1 * lk1, axis=-1))
    lam2 = np.exp(np.sum(lq2 * lk2, axis=-1))
    lam_full = (lam1 - lam2 + LAMBDA_INIT).astype(np.float32)  # (16,)

    in_maps = []
    for core in range(N_CORES):
        b, hg = core // 2, core % 2
        heads = np.arange(NH) + NH * hg  # global head idx
        wq = np.empty((C, C), np.float32)
        wk = np.empty((C, C), np.float32)
        wv = np.empty((C, C), np.float32)
        for h in range(NH):
            H = NH * hg + h
            wq[:, 128 * h : 128 * h + 64] = Wq1[:, HS * H : HS * (H + 1)]
            wq[:, 128 * h + 64 : 128 * (h + 1)] = Wq2[:, HS * H : HS * (H + 1)]
            wk[:, 128 * h : 128 * h + 64] = Wk1[:, HS * H : HS * (H + 1)]
            wk[:, 128 * h + 64 : 128 * (h + 1)] = Wk2[:, HS * H : HS * (H + 1)]
            wv[:, 128 * h : 128 * (h + 1)] = Wv[:, 128 * H : 128 * (H + 1)]
        wc = np.ascontiguousarray(Wc[1024 * hg : 1024 * (hg + 1), :])
        lamneg = np.broadcast_to(
            -lam_full[heads][None, :], (P, NH)
        ).astype(np.float32)
        in_maps.append(
            {
                "x": np.ascontiguousarray(x[b]),
                "wq": wq,
                "wk": wk,
                "wv": wv,
                "wc": wc,
                "lamneg": np.ascontiguousarray(lamneg),
            }
        )
    return in_maps


def run(inputs, trace=False, **kw):
    from concourse.bass_utils import run_bass_kernel_spmd

    nc = _get_program()
    in_maps = make_in_maps(inputs)
    res = run_bass_kernel_spmd(
        nc, in_maps, core_ids=list(range(N_CORES)), trace=trace, **kw
    )
    B = 4
    out = np.empty((B, T, C), np.float32)
    for b in range(B):
        out[b] = res.results[2 * b]["out"] + res.results[2 * b + 1]["out"]
    return out, res


def kernel(**inputs) -> np.ndarray:
    out, _ = run(inputs, trace=False)
    return out
